# revision 1
# baseline (speedup 1.0000x reference)
"""Trainium2 Bass kernel for nn_KernelToeplitzCausalLinear.

Computes, for x (B=8, E=2048, S=1024), weight (4, 1024), bias (1024,):

    out[b, e, t] = sum_k sum_{s<=t} x[b, e+k-3, s] * weight[k, t-s] + bias[t]

i.e. a causal 4-tap shift along E combined with a full causal (upper-
triangular Toeplitz) matmul along the dim axis.

Sharding: data-parallel over batch B -> one NeuronCore per batch element
(no halo: the E-shifts stay within a batch element).  The small weight is
replicated: host precomputes the 32 distinct 128x128 Toeplitz blocks
WB[k, d] (d = tblock - sblock) as strips WS[k] = [Z | B0 | ... | B7]
(128 x 1152, the leading zero block serves widened diagonal chunks).

Per-core kernel (Tile framework, fp32r matmuls at full PE rate):
  1. x is transposed on-chip (PE transpose, 128x128 tiles) into strips
     XT[sb] (128 x 2051) with 3 zero columns of left padding, so all four
     taps become free-dim offsets 128*j + k of one strip.
  2. For each 128-row e-tile j: 48 matmuls accumulate the block-sparse
     product into a 2-bank PSUM tile (stationary = XT slice, moving = WS
     slices with variable widths >= 256, skipping zero blocks below the
     Toeplitz diagonal); bias is added during the PSUM->SBUF copy.
  3. Transposes for e-tile j+1 are issued ahead of the matmuls of e-tile
     j so their PSUM->SBUF copies overlap the MM stream.
"""
import numpy as np
from contextlib import ExitStack

import concourse.bass as bass
import concourse.tile as tile
from concourse import bacc, mybir
from concourse.bass_utils import run_bass_kernel_spmd

P = 128
B = 8
E = 2048
S = 1024
K = 4
NB = S // P          # 8 s-blocks
NJ = E // P          # 16 e-tiles
F32 = mybir.dt.float32
F32R = mybir.dt.float32r

# per-sb list of (c0, c1) output-column chunks.  Chunks never straddle the
# 512-wide PSUM bank boundary and are widened to >= 256 columns (fp32r runs
# 4x slower below 256); widened chunks read the leading zero block of WS.
CHUNKS = {
    0: [(0, 512), (512, 1024)],
    1: [(128, 512), (512, 1024)],
    2: [(256, 512), (512, 1024)],
    3: [(256, 512), (512, 1024)],
    4: [(512, 1024)],
    5: [(640, 1024)],
    6: [(768, 1024)],
    7: [(768, 1024)],
}


def make_wstrips(weight: np.ndarray) -> np.ndarray:
    """(4, 1024) weight rows -> (4, 128, 1152) strips [Z|B0..B7] with
    WS[k, i, c] = weight[k, c - 128 - i] where valid, else 0."""
    offs = np.arange(9 * P)[None, :] - P - np.arange(P)[:, None]
    valid = (offs >= 0) & (offs < S)
    ws = np.where(valid[None], weight[:, offs.clip(0, S - 1)], 0.0)
    return np.ascontiguousarray(ws.astype(np.float32))


def build_nc(reps: int = 1):
    nc = bacc.Bacc("TRN2", target_bir_lowering=False, debug=False)
    x_d = nc.dram_tensor("x", [E, S], F32R, kind="ExternalInput").ap()
    w_d = nc.dram_tensor("ws", [K, P, 9 * P], F32R, kind="ExternalInput").ap()
    b_d = nc.dram_tensor("bias", [P, S], F32, kind="ExternalInput").ap()
    i_d = nc.dram_tensor("ident", [P, P], F32R, kind="ExternalInput").ap()
    o_d = nc.dram_tensor("out", [E, S], F32, kind="ExternalOutput").ap()

    with tile.TileContext(nc) as tc, ExitStack() as ctx:
        consts = ctx.enter_context(tc.tile_pool(name="consts", bufs=1))
        xt_pool = ctx.enter_context(tc.tile_pool(name="xt", bufs=1))
        ws_pool = ctx.enter_context(tc.tile_pool(name="wsp", bufs=1))
        xin_pool = ctx.enter_context(tc.tile_pool(name="xin", bufs=3))
        osb_pool = ctx.enter_context(tc.tile_pool(name="osb", bufs=3))
        tpsum = ctx.enter_context(tc.tile_pool(name="tpsum", bufs=4, space="PSUM"))
        opsum = ctx.enter_context(tc.tile_pool(name="opsum", bufs=4, space="PSUM"))

        ident = consts.tile([P, P], F32R)
        nc.sync.dma_start(ident[:], i_d[:])
        bias_rep = consts.tile([P, S], F32)
        nc.sync.dma_start(bias_rep[:], b_d[:])

        WS = []
        for k in range(K):
            t = ws_pool.tile([P, 9 * P], F32R, name=f"ws{k}")
            nc.sync.dma_start(t[:], w_d[k])
            WS.append(t)

        XT = []
        for sb in range(NB):
            t = xt_pool.tile([P, E + 3], F32R, name=f"xt{sb}")
            nc.vector.memset(t[:, 0:3].bitcast(F32), 0.0)
            XT.append(t)

        def trans_stage(j):
            xin = xin_pool.tile([P, S], F32R, name="xin")
            nc.sync.dma_start(xin[:], x_d[j * P:(j + 1) * P, :])
            for sb in range(NB):
                tp = tpsum.tile([P, P], F32R, name="tp")
                nc.tensor.transpose(tp[:], xin[:, sb * P:(sb + 1) * P], ident[:])
                nc.vector.tensor_copy(XT[sb][:, j * P + 3:(j + 1) * P + 3], tp[:])

        def body(_iv=None):
            trans_stage(0)
            for j in range(NJ):
                if j + 1 < NJ:
                    trans_stage(j + 1)

                pb = [opsum.tile([P, 512], F32, name="ob") for _ in range(2)]
                mms = []
                for k in range(K):
                    for sb in range(NB):
                        lhsT = XT[sb][:, P * j + k: P * j + k + P]
                        for (c0, c1) in CHUNKS[sb]:
                            bank = 1 if c0 >= 512 else 0
                            w0 = P + c0 - P * sb
                            rhs = WS[k][:, w0: w0 + (c1 - c0)]
                            outap = pb[bank][:, c0 - 512 * bank: c1 - 512 * bank]
                            mms.append((bank, outap, lhsT, rhs))
                seen = set()
                last_idx = {b: max(i for i, m in enumerate(mms) if m[0] == b)
                            for b in (0, 1)}
                for i, (bank, outap, lhsT, rhs) in enumerate(mms):
                    nc.tensor.matmul(
                        outap, lhsT, rhs,
                        start=bank not in seen,
                        stop=i == last_idx[bank],
                    )
                    seen.add(bank)

                osb = osb_pool.tile([P, S], F32, name="osb")
                for h in range(2):
                    nc.vector.tensor_add(
                        osb[:, h * 512:(h + 1) * 512], pb[h][:],
                        bias_rep[:, h * 512:(h + 1) * 512],
                    )
                nc.sync.dma_start(o_d[j * P:(j + 1) * P, :], osb[:])

        if reps == 1:
            body()
        else:
            with tc.For_i(0, reps, 1):
                body()

    nc.compile()
    return nc


_NC_CACHE = {}


def _get_nc():
    if 'nc' not in _NC_CACHE:
        _NC_CACHE['nc'] = build_nc(1)
    return _NC_CACHE['nc']


def kernel(x: np.ndarray, weight: np.ndarray, bias: np.ndarray) -> np.ndarray:
    x = np.ascontiguousarray(np.asarray(x, dtype=np.float32))
    weight = np.asarray(weight, dtype=np.float32)
    bias = np.asarray(bias, dtype=np.float32)
    assert x.shape == (B, E, S), x.shape
    assert weight.shape == (K, S), weight.shape
    assert bias.shape == (S,), bias.shape

    ws = make_wstrips(weight)
    ident = np.eye(P, dtype=np.float32)
    bias_rep = np.ascontiguousarray(
        np.broadcast_to(bias, (P, S)).astype(np.float32))
    in_maps = [
        {"x": np.ascontiguousarray(x[b]), "ws": ws,
         "bias": bias_rep, "ident": ident}
        for b in range(B)
    ]
    nc = _get_nc()
    res = run_bass_kernel_spmd(nc, in_maps, list(range(B)))
    out = np.stack([res.results[b]["out"] for b in range(B)]).astype(np.float32)
    return out



# revision 2
# speedup vs baseline: 1.2554x; 1.2554x over previous
"""Trainium2 Bass kernel for nn_KernelToeplitzCausalLinear.

Computes, for x (B=8, E=2048, S=1024), weight (4, 1024), bias (1024,):

    out[b, e, t] = sum_k sum_{s<=t} x[b, e+k-3, s] * weight[k, t-s] + bias[t]

i.e. a causal 4-tap shift along E combined with a full causal (upper-
triangular Toeplitz) matmul along the dim axis.

Sharding: data-parallel over batch B -> one NeuronCore per batch element
(no halo: the E-shifts stay within a batch element).  The small weight is
replicated: host precomputes the 32 distinct 128x128 Toeplitz blocks as
strips WS[k] = [Z | B0 | ... | B7] (128 x 1152, bf16).

v2 design (bf16 datapath, tol 2e-2 >> bf16's ~2e-3):
  1. Host casts x to bf16.  On-chip, x is transposed by the DMA XBAR
     (dma_start_transpose, 2-byte dtype) directly into SBUF strips
     XT[sb] (128 x 2080; data at col 32, 3 zero pad cols 29..31), in
     512-row chunks so the j-loop overlaps the loads.  No PE transposes,
     no PSUM->SBUF transpose copies.
  2. Per 128-row e-tile j: 48 bf16 matmuls (4 taps x 12 triangular
     chunks, exact 128-granularity -- bf16 has no >=256-column penalty)
     accumulate into a 2-bank PSUM tile; stationary = XT slice (shifted
     by tap k), moving = WS strips.  bf16 stationaries are FWL-eligible
     so LDWEIGHTS hides under the matmul stream.
  3. Bias is added during the PSUM->SBUF copy (DVE); fp32 out DMA.
"""
import numpy as np
from contextlib import ExitStack

import ml_dtypes

import concourse.bass as bass
import concourse.tile as tile
from concourse import bacc, mybir
from concourse.bass_utils import run_bass_kernel_spmd

P = 128
B = 8
E = 2048
S = 1024
K = 4
NB = S // P          # 8 s-blocks
NJ = E // P          # 16 e-tiles
PAD = 32             # strip data starts at col 32 (xbar-aligned); e=i -> col 32+i
ECH = 512            # e-rows per transposing DMA chunk
F32 = mybir.dt.float32
BF16 = mybir.dt.bfloat16

# per-sb list of (c0, c1) output-column chunks, exact 128-granular
# triangle, split at the 512-wide PSUM bank boundary.
CHUNKS = {
    0: [(0, 512), (512, 1024)],
    1: [(128, 512), (512, 1024)],
    2: [(256, 512), (512, 1024)],
    3: [(384, 512), (512, 1024)],
    4: [(512, 1024)],
    5: [(640, 1024)],
    6: [(768, 1024)],
    7: [(896, 1024)],
}


def make_wstrips(weight: np.ndarray) -> np.ndarray:
    """(4, 1024) weight rows -> (4, 128, 1152) strips [Z|B0..B7] with
    WS[k, i, c] = weight[k, c - 128 - i] where valid, else 0 (bf16)."""
    offs = np.arange(9 * P)[None, :] - P - np.arange(P)[:, None]
    valid = (offs >= 0) & (offs < S)
    ws = np.where(valid[None], weight[:, offs.clip(0, S - 1)], 0.0)
    return np.ascontiguousarray(ws.astype(ml_dtypes.bfloat16))


def build_nc(reps: int = 1):
    nc = bacc.Bacc("TRN2", target_bir_lowering=False, debug=False)
    x_d = nc.dram_tensor("x", [E, S], BF16, kind="ExternalInput").ap()
    w_d = nc.dram_tensor("ws", [K, P, 9 * P], BF16, kind="ExternalInput").ap()
    b_d = nc.dram_tensor("bias", [P, S], F32, kind="ExternalInput").ap()
    o_d = nc.dram_tensor("out", [E, S], F32, kind="ExternalOutput").ap()

    with tile.TileContext(nc) as tc, ExitStack() as ctx:
        consts = ctx.enter_context(tc.tile_pool(name="consts", bufs=1))
        xt_pool = ctx.enter_context(tc.tile_pool(name="xt", bufs=1))
        ws_pool = ctx.enter_context(tc.tile_pool(name="wsp", bufs=1))
        osb_pool = ctx.enter_context(tc.tile_pool(name="osb", bufs=3))
        opsum = ctx.enter_context(tc.tile_pool(name="opsum", bufs=4, space="PSUM"))

        bias_rep = consts.tile([P, S], F32)
        nc.sync.dma_start(bias_rep[:], b_d[:])

        WS = []
        for k in range(K):
            t = ws_pool.tile([P, 9 * P], BF16, name=f"ws{k}")
            nc.sync.dma_start(t[:], w_d[k])
            WS.append(t)

        XT = []
        for sb in range(NB):
            t = xt_pool.tile([P, PAD + E], BF16, name=f"xt{sb}")
            nc.vector.memset(t[:, PAD - 3:PAD], 0.0)
            XT.append(t)

        def body(_iv=None):
            # x.T strips via DMA xbar transpose, chunked along E
            for m in range(E // ECH):
                for sb in range(NB):
                    nc.sync.dma_start_transpose(
                        XT[sb][:, PAD + m * ECH: PAD + (m + 1) * ECH],
                        x_d[m * ECH:(m + 1) * ECH, sb * P:(sb + 1) * P],
                    )

            for j in range(NJ):
                pb = [opsum.tile([P, 512], F32, name="ob") for _ in range(2)]
                mms = []
                for k in range(K):
                    for sb in range(NB):
                        c = PAD + j * P + k - 3
                        lhsT = XT[sb][:, c: c + P]
                        for (c0, c1) in CHUNKS[sb]:
                            bank = 1 if c0 >= 512 else 0
                            w0 = P + c0 - P * sb
                            rhs = WS[k][:, w0: w0 + (c1 - c0)]
                            outap = pb[bank][:, c0 - 512 * bank: c1 - 512 * bank]
                            mms.append((bank, outap, lhsT, rhs))
                seen = set()
                last_idx = {b: max(i for i, m in enumerate(mms) if m[0] == b)
                            for b in (0, 1)}
                for i, (bank, outap, lhsT, rhs) in enumerate(mms):
                    nc.tensor.matmul(
                        outap, lhsT, rhs,
                        start=bank not in seen,
                        stop=i == last_idx[bank],
                    )
                    seen.add(bank)

                osb = osb_pool.tile([P, S], F32, name="osb")
                for h in range(2):
                    nc.vector.tensor_add(
                        osb[:, h * 512:(h + 1) * 512], pb[h][:],
                        bias_rep[:, h * 512:(h + 1) * 512],
                    )
                nc.sync.dma_start(o_d[j * P:(j + 1) * P, :], osb[:])

        if reps == 1:
            body()
        else:
            with tc.For_i(0, reps, 1):
                body()

    nc.compile()
    return nc


def make_inmaps(x: np.ndarray, weight: np.ndarray, bias: np.ndarray):
    x = np.asarray(x, dtype=np.float32)
    weight = np.asarray(weight, dtype=np.float32)
    bias = np.asarray(bias, dtype=np.float32)
    assert x.shape == (B, E, S), x.shape
    assert weight.shape == (K, S), weight.shape
    assert bias.shape == (S,), bias.shape
    ws = make_wstrips(weight)
    bias_rep = np.ascontiguousarray(
        np.broadcast_to(bias, (P, S)).astype(np.float32))
    xb = np.ascontiguousarray(x.astype(ml_dtypes.bfloat16))
    return [
        {"x": xb[b], "ws": ws, "bias": bias_rep}
        for b in range(B)
    ]


_NC_CACHE = {}


def _get_nc():
    if 'nc' not in _NC_CACHE:
        _NC_CACHE['nc'] = build_nc(1)
    return _NC_CACHE['nc']


def kernel(x: np.ndarray, weight: np.ndarray, bias: np.ndarray) -> np.ndarray:
    in_maps = make_inmaps(x, weight, bias)
    nc = _get_nc()
    res = run_bass_kernel_spmd(nc, in_maps, list(range(B)))
    out = np.stack([res.results[b]["out"] for b in range(B)]).astype(np.float32)
    return out
